# revision 2
# baseline (speedup 1.0000x reference)
"""External Attention (nn_External_Attention) on 8 TRN2 NeuronCores.

kernel(x, Wk, Wv) -> x + Wv @ l1norm_M(softmax_N(Wk @ x))
  x  [16, 512, 4096] f32,  Wk [256, 512] f32,  Wv [512, 256] f32

Sharding: data-parallel over batch B=16 -> 2 batches per core across 8 cores.
Each core runs an identical Bass/Tile program on its batch shard; results are
concatenated on host.

v2 design (vs the 224us f32r baseline):
  - All I/O in bf16: x is downcast on host (8.4 MB/core in), y is produced in
    bf16 on device (8.4 MB/core out) and upcast on host. Halves HBM traffic
    to ~47us of DMA at line rate. Precision budget allows it: |out| ~ 0.006|x|
    so attention-path errors are damped ~150x, and bf16 rounding of x/y gives
    ~0.3% L2 error vs the 2e-2 gate.
  - MM1/MM2/cs all bf16 (full PE rate, same as f32r, half the SBUF).
  - exp on ACT in [128,1024] tiles (amortizes the 352-cycle ramp).
  - 1/colsum via DVE reciprocal_approx_fast (no ACT Reciprocal -> no
    exp<->reciprocal table-set thrash; baseline paid 6 ACT_TABLE_LOADs).
  - residual add y = po + x split across engines: most tiles as a direct
    DVE tensor_tensor add from PSUM; the rest as ACT Copy (PSUM->SBUF bf16)
    + GPSIMD bf16 add, to keep every elementwise engine under the PE bound.
  - Phase B of batch 0 is interleaved with phase A of batch 1 so PE stays
    warm and DMA/ACT/DVE/GPSIMD overlap.
"""
from contextlib import ExitStack

import ml_dtypes
import numpy as np

import concourse.bacc as bacc
import concourse.mybir as mybir
import concourse.tile as tile
from concourse.bass_utils import run_bass_kernel_spmd

F32 = mybir.dt.float32
BF16 = mybir.dt.bfloat16
AF = mybir.ActivationFunctionType
ALU = mybir.AluOpType
AX = mybir.AxisListType

BF16NP = ml_dtypes.bfloat16

B, C, M, N = 16, 512, 256, 4096
NCORES = 8
BPC = B // NCORES
KC = C // 128   # 4 c-blocks
KM = M // 128   # 2 m-blocks
NT = 512        # matmul / phase-B tile width
NJ = N // NT    # 8
XT = 1024       # exp tile width
NJ2 = N // XT   # 4

# of the 2*NJ*KC = 64 residual-add tiles, this many per 8 go direct-DVE
# (PSUM f32 + bf16 x -> bf16 y, 1x mode); the rest go ACT-copy + GPSIMD-add.
DVE_ADD_PER8 = 5


def _build(nc):
    x_d = nc.dram_tensor("x", [BPC, C, N], BF16, kind="ExternalInput").ap()
    wkT_d = nc.dram_tensor("wkT", [C, M], BF16, kind="ExternalInput").ap()
    wvT_d = nc.dram_tensor("wvT", [M, C], BF16, kind="ExternalInput").ap()
    y_d = nc.dram_tensor("y", [BPC, C, N], BF16, kind="ExternalOutput").ap()

    with tile.TileContext(nc) as tc, ExitStack() as ctx:
        wpool = ctx.enter_context(tc.tile_pool(name="w", bufs=1))
        xpool = ctx.enter_context(tc.tile_pool(name="xp", bufs=2 * KC))
        epool = ctx.enter_context(tc.tile_pool(name="ep", bufs=2 * KM))
        ypool = ctx.enter_context(tc.tile_pool(name="yp", bufs=2 * KC))
        spool = ctx.enter_context(tc.tile_pool(name="sp", bufs=10))
        wvp_pool = ctx.enter_context(tc.tile_pool(name="wvp", bufs=2 * KM))
        rcpool = ctx.enter_context(tc.tile_pool(name="rc", bufs=6))
        bcpool = ctx.enter_context(tc.tile_pool(name="bcp", bufs=4))
        eppool = ctx.enter_context(tc.tile_pool(name="epp", bufs=6))
        evpool = ctx.enter_context(tc.tile_pool(name="ev", bufs=6))
        ps_l = ctx.enter_context(tc.tile_pool(name="ps_l", bufs=2, space="PSUM"))
        ps_cs = ctx.enter_context(tc.tile_pool(name="ps_cs", bufs=2, space="PSUM"))
        ps_o = ctx.enter_context(tc.tile_pool(name="ps_o", bufs=2, space="PSUM"))

        wk_sb = []
        for kc in range(KC):
            t = wpool.tile([128, M], BF16, tag=f"wk{kc}", name=f"wk{kc}")
            nc.sync.dma_start(t[:], wkT_d[kc * 128:(kc + 1) * 128, :])
            wk_sb.append(t)
        wv_sb = []
        for km in range(KM):
            t = wpool.tile([128, C], BF16, tag=f"wv{km}", name=f"wv{km}")
            nc.sync.dma_start(t[:], wvT_d[km * 128:(km + 1) * 128, :])
            wv_sb.append(t)

        X, E, RSP, RRB, WVP, Y = {}, {}, {}, {}, {}, {}
        ev_idx = [0]

        def load_x(b, halves):
            X[b] = []
            for kc in range(KC):
                t = xpool.tile([128, N], BF16, tag="x", name=f"x{b}_{kc}")
                src = x_d[b, kc * 128:(kc + 1) * 128, :]
                if halves:
                    h = N // 2
                    nc.sync.dma_start(t[:, 0:h], src[:, 0:h])
                    nc.sync.dma_start(t[:, h:N], src[:, h:N])
                else:
                    nc.sync.dma_start(t[:], src)
                X[b].append(t)

        def init_batch(b):
            E[b] = [epool.tile([128, N], BF16, tag="e", name=f"e{b}_{km}")
                    for km in range(KM)]
            RSP[b] = [spool.tile([128, NJ2], F32, tag="rsp", name=f"rsp{b}_{km}")
                      for km in range(KM)]
            Y[b] = [ypool.tile([128, N], BF16, tag="y", name=f"y{b}_{co}")
                    for co in range(KC)]

        def emit_A(b, jj):
            # MM1 + exp for columns [jj*XT, (jj+1)*XT)
            for km in range(KM):
                pl = ps_l.tile([128, XT], F32, tag="pl", name=f"pl{b}_{jj}_{km}")
                for h in range(XT // NT):
                    for kc in range(KC):
                        nc.tensor.matmul(
                            pl[:, h * NT:(h + 1) * NT],
                            wk_sb[kc][:, km * 128:(km + 1) * 128],
                            X[b][kc][:, jj * XT + h * NT: jj * XT + (h + 1) * NT],
                            start=(kc == 0), stop=(kc == KC - 1))
                nc.scalar.activation(
                    E[b][km][:, jj * XT:(jj + 1) * XT], pl[:],
                    AF.Exp, accum_out=RSP[b][km][:, jj:jj + 1])

        def emit_stats(b):
            RRB[b], WVP[b] = [], []
            for km in range(KM):
                rs = spool.tile([128, 1], F32, tag="rs", name=f"rs{b}_{km}")
                nc.vector.tensor_reduce(rs[:], RSP[b][km][:], axis=AX.X, op=ALU.add)
                rr = spool.tile([128, 1], F32, tag="rr", name=f"rr{b}_{km}")
                nc.vector.reciprocal(rr[:], rs[:])
                rrb = spool.tile([128, 1], BF16, tag="rrb", name=f"rrb{b}_{km}")
                nc.vector.tensor_copy(rrb[:], rr[:])
                RRB[b].append(rrb)
                t = wvp_pool.tile([128, C], BF16, tag="wvp", name=f"wvp{b}_{km}")
                nc.vector.tensor_scalar_mul(t[:], wv_sb[km][:], rr[:])
                WVP[b].append(t)

        def emit_B(b, j):
            sl = slice(j * NT, (j + 1) * NT)
            cs = ps_cs.tile([1, NT], F32, tag="cs", name=f"cs{b}_{j}")
            for km in range(KM):
                nc.tensor.matmul(cs[:], RRB[b][km][:], E[b][km][:, sl],
                                 start=(km == 0), stop=(km == KM - 1))
            rcs = rcpool.tile([1, NT], F32, tag="rcs", name=f"rcs{b}_{j}")
            nc.vector.reciprocal_approx_fast(rcs[:], cs[:])
            rcsb = rcpool.tile([1, NT], BF16, tag="rcsb", name=f"rcsb{b}_{j}")
            nc.scalar.activation(rcsb[:], rcs[:], AF.Copy)
            bc = bcpool.tile([128, NT], BF16, tag="bc", name=f"bc{b}_{j}")
            nc.gpsimd.partition_broadcast(bc[:], rcsb[:])
            ep_t = []
            for km in range(KM):
                t = eppool.tile([128, NT], BF16, tag="epp", name=f"epp{b}_{j}_{km}")
                nc.vector.tensor_tensor(t[:], E[b][km][:, sl], bc[:], op=ALU.mult)
                ep_t.append(t)
            for co in range(KC):
                po = ps_o.tile([128, NT], F32, tag="po", name=f"po{b}_{j}_{co}")
                for km in range(KM):
                    nc.tensor.matmul(po[:], WVP[b][km][:, co * 128:(co + 1) * 128],
                                     ep_t[km][:],
                                     start=(km == 0), stop=(km == KM - 1))
                ys = Y[b][co][:, sl]
                xs = X[b][co][:, sl]
                if ev_idx[0] % 8 < DVE_ADD_PER8:
                    nc.vector.tensor_tensor(ys, po[:], xs, op=ALU.add)
                else:
                    t = evpool.tile([128, NT], BF16, tag="ev", name=f"ev{b}_{j}_{co}")
                    nc.scalar.activation(t[:], po[:], AF.Copy)
                    nc.gpsimd.tensor_tensor(ys, t[:], xs, op=ALU.add)
                ev_idx[0] += 1

        def emit_store(b):
            for co in range(KC):
                nc.sync.dma_start(y_d[b, co * 128:(co + 1) * 128, :], Y[b][co][:])

        # ---- program ----
        load_x(0, halves=True)
        load_x(1, halves=False)
        init_batch(0)
        init_batch(1)
        for jj in range(NJ2):
            emit_A(0, jj)
        emit_stats(0)
        for j in range(NJ):
            emit_B(0, j)
            if j % 2 == 1:
                emit_A(1, j // 2)
        emit_store(0)
        emit_stats(1)
        for j in range(NJ):
            emit_B(1, j)
        emit_store(1)
    return nc


_CACHE = {}


def _get_program():
    if "nc" not in _CACHE:
        nc = bacc.Bacc("TRN2", target_bir_lowering=False, debug=False,
                       enable_asserts=True)
        _build(nc)
        nc.compile()
        _CACHE["nc"] = nc
    return _CACHE["nc"]


def _in_maps(x, Wk, Wv):
    x = np.asarray(x, dtype=np.float32)
    xb = np.ascontiguousarray(x).astype(BF16NP)
    wkT = np.ascontiguousarray(np.asarray(Wk, np.float32).T).astype(BF16NP)
    wvT = np.ascontiguousarray(np.asarray(Wv, np.float32).T).astype(BF16NP)
    return [{"x": xb[i * BPC:(i + 1) * BPC], "wkT": wkT, "wvT": wvT}
            for i in range(NCORES)]


def kernel(x, Wk, Wv):
    nc = _get_program()
    res = run_bass_kernel_spmd(nc, _in_maps(x, Wk, Wv), list(range(NCORES)))
    y = np.concatenate([res.results[i]["y"].astype(np.float32)
                        for i in range(NCORES)], axis=0)
    return np.ascontiguousarray(y)


# revision 5
# speedup vs baseline: 1.8799x; 1.8799x over previous
"""External Attention (nn_External_Attention) on 8 TRN2 NeuronCores.

kernel(x, Wk, Wv) -> x + Wv @ l1norm_M(softmax_N(Wk @ x))
  x  [16, 512, 4096] f32,  Wk [256, 512] f32,  Wv [512, 256] f32

Sharding: data-parallel over batch B=16 -> 2 batches per core across 8 cores.

v2b design notes:
  - All I/O bf16 (halves HBM traffic vs the f32r baseline); bf16 matmuls.
  - 1/colsum via DVE reciprocal_approx_fast (no ACT table thrash).
  - Anti-convoy structure: the per-column-tile normalizer chain
    (cs matmul -> reciprocal -> bf16 cast -> partition broadcast -> E*bc)
    is emitted stage-batched per batch (all 8 reciprocals back-to-back, then
    all casts, ...) so each strict-FIFO engine queue drains without
    head-of-line blocking; the consume loop (MM2 + residual add + store) then
    runs dense on the PE. Phase A of batch 1 is interleaved to cover batch
    0's prep, keeping the PE HAM-warm.
  - Residual add y = po + x split across engines (DVE direct-from-PSUM for
    6/8 tiles, ACT copy + GPSIMD bf16 add for 2/8).
"""
from contextlib import ExitStack

import ml_dtypes
import numpy as np

import concourse.bacc as bacc
import concourse.mybir as mybir
import concourse.tile as tile
from concourse.bass_utils import run_bass_kernel_spmd

F32 = mybir.dt.float32
BF16 = mybir.dt.bfloat16
AF = mybir.ActivationFunctionType
ALU = mybir.AluOpType
AX = mybir.AxisListType

BF16NP = ml_dtypes.bfloat16

B, C, M, N = 16, 512, 256, 4096
NCORES = 8
BPC = B // NCORES
KC = C // 128   # 4 c-blocks
KM = M // 128   # 2 m-blocks
NT = 512        # matmul tile width
NJ = N // NT    # 8
XT = 1024       # exp / E' tile width
NJ2 = N // XT   # 4

DVE_ADD_PER8 = 6  # of each 8 residual tiles, this many direct-DVE; rest ACT+GPS


def _build(nc):
    x_d = nc.dram_tensor("x", [BPC, C, N], BF16, kind="ExternalInput").ap()
    wkT_d = nc.dram_tensor("wkT", [C, M], BF16, kind="ExternalInput").ap()
    wvT_d = nc.dram_tensor("wvT", [M, C], BF16, kind="ExternalInput").ap()
    y_d = nc.dram_tensor("y", [BPC, C, N], BF16, kind="ExternalOutput").ap()

    with tile.TileContext(nc) as tc, ExitStack() as ctx:
        wpool = ctx.enter_context(tc.tile_pool(name="w", bufs=1))
        xpool = ctx.enter_context(tc.tile_pool(name="xp", bufs=2 * KC))
        epool = ctx.enter_context(tc.tile_pool(name="ep", bufs=2 * KM))
        spool = ctx.enter_context(tc.tile_pool(name="sp", bufs=10))
        wvp_pool = ctx.enter_context(tc.tile_pool(name="wvp", bufs=2 * KM))
        rcpool = ctx.enter_context(tc.tile_pool(name="rc", bufs=12))
        bcpool = ctx.enter_context(tc.tile_pool(name="bcp", bufs=2 * NJ2))
        eppool = ctx.enter_context(tc.tile_pool(name="epp", bufs=2 * KM))
        evpool = ctx.enter_context(tc.tile_pool(name="ev", bufs=6))
        ps_l = ctx.enter_context(tc.tile_pool(name="ps_l", bufs=2, space="PSUM"))
        ps_cs = ctx.enter_context(tc.tile_pool(name="ps_cs", bufs=2, space="PSUM"))
        ps_o = ctx.enter_context(tc.tile_pool(name="ps_o", bufs=2, space="PSUM"))

        wk_sb = []
        for kc in range(KC):
            t = wpool.tile([128, M], BF16, tag=f"wk{kc}", name=f"wk{kc}")
            nc.sync.dma_start(t[:], wkT_d[kc * 128:(kc + 1) * 128, :])
            wk_sb.append(t)
        wv_sb = []
        for km in range(KM):
            t = wpool.tile([128, C], BF16, tag=f"wv{km}", name=f"wv{km}")
            nc.sync.dma_start(t[:], wvT_d[km * 128:(km + 1) * 128, :])
            wv_sb.append(t)

        X, E, RSP, RRB, WVP, Y, CS, RCSB, BC, EP = ({} for _ in range(10))
        ev_idx = [0]

        def load_x(b, quarters):
            X[b] = [xpool.tile([128, N], BF16, tag="x", name=f"x{b}_{kc}")
                    for kc in range(KC)]
            if quarters:
                # column-major emission so the leading columns of every
                # kc-block land first and MM1 can start early
                q = N // 4
                for h in range(4):
                    for kc in range(KC):
                        nc.sync.dma_start(
                            X[b][kc][:, h * q:(h + 1) * q],
                            x_d[b, kc * 128:(kc + 1) * 128, h * q:(h + 1) * q])
            else:
                for kc in range(KC):
                    nc.sync.dma_start(X[b][kc][:],
                                      x_d[b, kc * 128:(kc + 1) * 128, :])

        def init_batch(b):
            E[b] = [epool.tile([128, N], BF16, tag="e", name=f"e{b}_{km}")
                    for km in range(KM)]
            RSP[b] = [spool.tile([128, NJ2], F32, tag="rsp", name=f"rsp{b}_{km}")
                      for km in range(KM)]
            # y = x + out is written IN PLACE into the x tiles (x's last
            # reader is this very add), so no separate y staging is needed.
            Y[b] = X[b]
            EP[b] = [eppool.tile([128, N], BF16, tag="epp", name=f"epp{b}_{km}")
                     for km in range(KM)]
            CS[b], RCSB[b], BC[b] = [], [], []

        def emit_A(b, jj):
            # MM1 + exp for columns [jj*XT, (jj+1)*XT)
            for km in range(KM):
                pl = ps_l.tile([128, XT], F32, tag="pl", name=f"pl{b}_{jj}_{km}")
                for h in range(XT // NT):
                    for kc in range(KC):
                        nc.tensor.matmul(
                            pl[:, h * NT:(h + 1) * NT],
                            wk_sb[kc][:, km * 128:(km + 1) * 128],
                            X[b][kc][:, jj * XT + h * NT: jj * XT + (h + 1) * NT],
                            start=(kc == 0), stop=(kc == KC - 1))
                nc.scalar.activation(
                    E[b][km][:, jj * XT:(jj + 1) * XT], pl[:],
                    AF.Exp, accum_out=RSP[b][km][:, jj:jj + 1])

        def emit_stats(b):
            RRB[b], WVP[b] = [], []
            for km in range(KM):
                rs = spool.tile([128, 1], F32, tag="rs", name=f"rs{b}_{km}")
                nc.vector.tensor_reduce(rs[:], RSP[b][km][:], axis=AX.X, op=ALU.add)
                rr = spool.tile([128, 1], F32, tag="rr", name=f"rr{b}_{km}")
                nc.vector.reciprocal(rr[:], rs[:])
                rrb = spool.tile([128, 1], BF16, tag="rrb", name=f"rrb{b}_{km}")
                nc.vector.tensor_copy(rrb[:], rr[:])
                RRB[b].append(rrb)
                t = wvp_pool.tile([128, C], BF16, tag="wvp", name=f"wvp{b}_{km}")
                nc.vector.tensor_scalar_mul(t[:], wv_sb[km][:], rr[:])
                WVP[b].append(t)

        # ---- stage-batched normalizer prep (per batch) ----
        def emit_cs(b):
            for j in range(NJ):
                cs = ps_cs.tile([1, NT], F32, tag="cs", name=f"cs{b}_{j}")
                for km in range(KM):
                    nc.tensor.matmul(cs[:], RRB[b][km][:],
                                     E[b][km][:, j * NT:(j + 1) * NT],
                                     start=(km == 0), stop=(km == KM - 1))
                CS[b].append(cs)

        def emit_recip(b):
            for j in range(NJ):
                rcs = rcpool.tile([1, NT], F32, tag="rcs", name=f"rcs{b}_{j}")
                nc.vector.reciprocal_approx_fast(rcs[:], CS[b][j][:])
                rcsb = rcpool.tile([1, NT], BF16, tag="rcsb", name=f"rcsb{b}_{j}")
                RCSB[b].append((rcs, rcsb))

        def emit_cast(b):
            for j in range(NJ):
                rcs, rcsb = RCSB[b][j]
                nc.scalar.activation(rcsb[:], rcs[:], AF.Copy)

        def emit_bcast(b):
            for j2 in range(NJ2):
                bc = bcpool.tile([128, XT], BF16, tag="bc", name=f"bc{b}_{j2}")
                for h in range(XT // NT):
                    nc.gpsimd.partition_broadcast(
                        bc[:, h * NT:(h + 1) * NT], RCSB[b][2 * j2 + h][1][:])
                BC[b].append(bc)

        def emit_epmul(b):
            for j2 in range(NJ2):
                sl = slice(j2 * XT, (j2 + 1) * XT)
                for km in range(KM):
                    nc.vector.tensor_tensor(EP[b][km][:, sl], E[b][km][:, sl],
                                            BC[b][j2][:], op=ALU.mult)

        def emit_consume(b, j):
            sl = slice(j * NT, (j + 1) * NT)
            for co in range(KC):
                po = ps_o.tile([128, NT], F32, tag="po", name=f"po{b}_{j}_{co}")
                for km in range(KM):
                    nc.tensor.matmul(po[:], WVP[b][km][:, co * 128:(co + 1) * 128],
                                     EP[b][km][:, sl],
                                     start=(km == 0), stop=(km == KM - 1))
                ys = Y[b][co][:, sl]
                xs = X[b][co][:, sl]
                if ev_idx[0] % 8 < DVE_ADD_PER8:
                    nc.vector.tensor_tensor(ys, po[:], xs, op=ALU.add)
                else:
                    t = evpool.tile([128, NT], BF16, tag="ev", name=f"ev{b}_{j}_{co}")
                    nc.scalar.activation(t[:], po[:], AF.Copy)
                    nc.gpsimd.tensor_tensor(ys, t[:], xs, op=ALU.add)
                ev_idx[0] += 1

        def emit_store(b, cos):
            for co in cos:
                nc.sync.dma_start(y_d[b, co * 128:(co + 1) * 128, :], Y[b][co][:])

        # ---- program ----
        load_x(0, quarters=True)
        load_x(1, quarters=False)
        init_batch(0)
        init_batch(1)
        for jj in range(NJ2):
            emit_A(0, jj)
        emit_stats(0)
        # batch-0 normalizer prep, stage-batched; PE continues into phase A of
        # batch 1 while DVE/ACT/GPS drain the prep stages.
        emit_cs(0)
        emit_recip(0)
        emit_cast(0)
        emit_bcast(0)
        for jj in range(NJ2):
            emit_A(1, jj)
        emit_epmul(0)
        # consume batch 0; batch-1 stats+prep emitted early so its chain
        # drains on the small engines while the PE runs batch-0 MM2s.
        emit_consume(0, 0)
        emit_consume(0, 1)
        emit_stats(1)
        emit_cs(1)
        emit_recip(1)
        emit_cast(1)
        emit_bcast(1)
        for j in range(2, NJ):
            emit_consume(0, j)
        emit_epmul(1)
        emit_store(0, range(KC))
        for j in range(NJ):
            emit_consume(1, j)
        emit_store(1, range(KC))
    return nc


_CACHE = {}


def _get_program():
    if "nc" not in _CACHE:
        nc = bacc.Bacc("TRN2", target_bir_lowering=False, debug=False,
                       enable_asserts=True)
        _build(nc)
        nc.compile()
        _CACHE["nc"] = nc
    return _CACHE["nc"]


def _in_maps(x, Wk, Wv):
    x = np.asarray(x, dtype=np.float32)
    xb = np.ascontiguousarray(x).astype(BF16NP)
    wkT = np.ascontiguousarray(np.asarray(Wk, np.float32).T).astype(BF16NP)
    wvT = np.ascontiguousarray(np.asarray(Wv, np.float32).T).astype(BF16NP)
    return [{"x": xb[i * BPC:(i + 1) * BPC], "wkT": wkT, "wvT": wvT}
            for i in range(NCORES)]


def kernel(x, Wk, Wv):
    nc = _get_program()
    res = run_bass_kernel_spmd(nc, _in_maps(x, Wk, Wv), list(range(NCORES)))
    y = np.concatenate([res.results[i]["y"].astype(np.float32)
                        for i in range(NCORES)], axis=0)
    return np.ascontiguousarray(y)


# revision 7
# speedup vs baseline: 1.9363x; 1.0300x over previous
"""External Attention (nn_External_Attention) on 8 TRN2 NeuronCores.

kernel(x, Wk, Wv) -> x + Wv @ l1norm_M(softmax_N(Wk @ x))
  x  [16, 512, 4096] f32,  Wk [256, 512] f32,  Wv [512, 256] f32

Sharding: data-parallel over batch B=16 -> 2 batches per core across 8 cores.

v2c design notes (on top of v2b's stage-batched anti-convoy structure):
  - All I/O bf16; bf16 matmuls; y written in place into the x tiles.
  - Lead-in: packed weight DMAs (1 issue each), batch-0 x loaded via 4
    kc-spanning 1MB quarter DMAs, batch-1 x via one 4MB DMA; 16 warmup
    matmuls on a memset tile keep the PE HAM-warm through the load window.
  - Normalizer chain per batch, stage-batched: cs matmuls ([1,1024] PSUM)
    -> DVE reciprocal_approx_fast -> ACT bf16 cast -> GPSIMD partition
    broadcast (the only GPSIMD op type -> its Q7 library loads once).
  - E' = E * bc on DVE at [128,2048] (2x bf16 mode).
  - Consume loop co-major with j-paired matmuls (stationary operand reused
    back-to-back); y stores per co-half so the last store is only 512KB.
  - Residual adds alternate: even tiles direct DVE (PSUM f32 + x -> bf16),
    odd tiles ACT copy to bf16 + DVE bf16 add.
"""
from contextlib import ExitStack

import ml_dtypes
import numpy as np

import concourse.bacc as bacc
import concourse.mybir as mybir
import concourse.tile as tile
from concourse.bass_utils import run_bass_kernel_spmd

F32 = mybir.dt.float32
BF16 = mybir.dt.bfloat16
AF = mybir.ActivationFunctionType
ALU = mybir.AluOpType
AX = mybir.AxisListType

BF16NP = ml_dtypes.bfloat16

B, C, M, N = 16, 512, 256, 4096
NCORES = 8
BPC = B // NCORES
KC = C // 128   # 4 c-blocks
KM = M // 128   # 2 m-blocks
NT = 512        # matmul tile width
NJ = N // NT    # 8
XT = 1024       # exp / cs tile width
NJ2 = N // XT   # 4
WT = 2048       # E' / bc tile width
NW = N // WT    # 2
NWARM = 16


def _build(nc):
    x_d = nc.dram_tensor("x", [BPC, C, N], BF16, kind="ExternalInput").ap()
    wkT_d = nc.dram_tensor("wkT", [C, M], BF16, kind="ExternalInput").ap()
    wvT_d = nc.dram_tensor("wvT", [M, C], BF16, kind="ExternalInput").ap()
    y_d = nc.dram_tensor("y", [BPC, C, N], BF16, kind="ExternalOutput").ap()

    with tile.TileContext(nc) as tc, ExitStack() as ctx:
        wpool = ctx.enter_context(tc.tile_pool(name="w", bufs=1))
        xpool = ctx.enter_context(tc.tile_pool(name="xp", bufs=2))
        epool = ctx.enter_context(tc.tile_pool(name="ep", bufs=2 * KM))
        spool = ctx.enter_context(tc.tile_pool(name="sp", bufs=10))
        wvp_pool = ctx.enter_context(tc.tile_pool(name="wvp", bufs=2 * KM))
        rcpool = ctx.enter_context(tc.tile_pool(name="rc", bufs=4))
        bcpool = ctx.enter_context(tc.tile_pool(name="bcp", bufs=2 * NW))
        eppool = ctx.enter_context(tc.tile_pool(name="epp", bufs=2 * KM))
        evpool = ctx.enter_context(tc.tile_pool(name="ev", bufs=6))
        ps_l = ctx.enter_context(tc.tile_pool(name="ps_l", bufs=2, space="PSUM"))
        ps_cs = ctx.enter_context(tc.tile_pool(name="ps_cs", bufs=1, space="PSUM"))
        ps_o = ctx.enter_context(tc.tile_pool(name="ps_o", bufs=2, space="PSUM"))

        # PE warmup: 16 matmuls on a zeroed tile keep HAM busy while x loads.
        wz = wpool.tile([128, NT], BF16, tag="wz", name="wz")
        nc.vector.memset(wz[:], 0)
        for i in range(NWARM):
            po = ps_o.tile([128, NT], F32, tag="po", name=f"warm{i}")
            nc.tensor.matmul(po[:], wz[:, 0:128], wz[:], start=True, stop=True)

        # packed weight loads: one DMA each
        wk_sb = wpool.tile([128, KC * M], BF16, tag="wk", name="wk")
        nc.sync.dma_start(wk_sb[:].rearrange("p (k m) -> p k m", k=KC),
                          wkT_d.rearrange("(k p) m -> p k m", k=KC))
        wv_sb = wpool.tile([128, KM * C], BF16, tag="wv", name="wv")
        nc.scalar.dma_start(wv_sb[:].rearrange("p (k m) -> p k m", k=KM),
                            wvT_d.rearrange("(k p) m -> p k m", k=KM))

        def wk_ap(kc, km):
            return wk_sb[:, kc * M + km * 128: kc * M + (km + 1) * 128]

        X, E, RSP, RRB, WVP, CS, RCS, BC, EP = ({} for _ in range(9))
        ev_idx = [0]

        # x tiles: one [128, KC*N] tile per batch; kc block at [:, kc*N:+N]
        for b in range(BPC):
            X[b] = xpool.tile([128, KC * N], BF16, tag="x", name=f"x{b}")
        src0 = x_d[0].rearrange("(k p) n -> p k n", k=KC)
        dst0 = X[0][:].rearrange("p (k n) -> p k n", k=KC)
        q = N // 4
        for h in range(4):
            nc.sync.dma_start(dst0[:, :, h * q:(h + 1) * q],
                              src0[:, :, h * q:(h + 1) * q])
        nc.sync.dma_start(X[1][:].rearrange("p (k n) -> p k n", k=KC),
                          x_d[1].rearrange("(k p) n -> p k n", k=KC))

        def xs(b, kc, sl):
            return X[b][:, kc * N + sl.start: kc * N + sl.stop]

        def init_batch(b):
            E[b] = [epool.tile([128, N], BF16, tag="e", name=f"e{b}_{km}")
                    for km in range(KM)]
            RSP[b] = [spool.tile([128, NJ2], F32, tag="rsp", name=f"rsp{b}_{km}")
                      for km in range(KM)]
            EP[b] = [eppool.tile([128, N], BF16, tag="epp", name=f"epp{b}_{km}")
                     for km in range(KM)]
            CS[b], RCS[b], BC[b] = [], [], []

        def emit_A(b, jj):
            # MM1 + exp for columns [jj*XT, (jj+1)*XT); kc outer so the
            # stationary operand is reused across the two h halves.
            for km in range(KM):
                pl = ps_l.tile([128, XT], F32, tag="pl", name=f"pl{b}_{jj}_{km}")
                for kc in range(KC):
                    for h in range(XT // NT):
                        nc.tensor.matmul(
                            pl[:, h * NT:(h + 1) * NT],
                            wk_ap(kc, km),
                            xs(b, kc, slice(jj * XT + h * NT,
                                            jj * XT + (h + 1) * NT)),
                            start=(kc == 0), stop=(kc == KC - 1))
                nc.scalar.activation(
                    E[b][km][:, jj * XT:(jj + 1) * XT], pl[:],
                    AF.Exp, accum_out=RSP[b][km][:, jj:jj + 1])

        def emit_stats(b):
            RRB[b], WVP[b] = [], []
            for km in range(KM):
                rs = spool.tile([128, 1], F32, tag="rs", name=f"rs{b}_{km}")
                nc.vector.tensor_reduce(rs[:], RSP[b][km][:], axis=AX.X, op=ALU.add)
                rr = spool.tile([128, 1], F32, tag="rr", name=f"rr{b}_{km}")
                nc.vector.reciprocal(rr[:], rs[:])
                rrb = spool.tile([128, 1], BF16, tag="rrb", name=f"rrb{b}_{km}")
                nc.vector.tensor_copy(rrb[:], rr[:])
                RRB[b].append(rrb)
                t = wvp_pool.tile([128, C], BF16, tag="wvp", name=f"wvp{b}_{km}")
                nc.vector.tensor_scalar_mul(t[:], wv_sb[:, km * C:(km + 1) * C], rr[:])
                WVP[b].append(t)

        def emit_cs(b, j2):
            # colsum for columns [j2*XT, (j2+1)*XT) + reciprocal + bf16 cast
            cs = ps_cs.tile([1, XT], F32, tag="cs", name=f"cs{b}_{j2}")
            for km in range(KM):
                for h in range(XT // NT):
                    nc.tensor.matmul(
                        cs[:, h * NT:(h + 1) * NT], RRB[b][km][:],
                        E[b][km][:, j2 * XT + h * NT: j2 * XT + (h + 1) * NT],
                        start=(km == 0), stop=(km == KM - 1))
            rcs = rcpool.tile([1, XT], F32, tag="rcs", name=f"rcs{b}_{j2}")
            nc.vector.reciprocal_approx_fast(rcs[:], cs[:])
            rcsb = rcpool.tile([1, XT], BF16, tag="rcsb", name=f"rcsb{b}_{j2}")
            nc.scalar.activation(rcsb[:], rcs[:], AF.Copy)
            RCS[b].append(rcsb)

        def emit_bcast(b, w):
            bc = bcpool.tile([128, WT], BF16, tag="bc", name=f"bc{b}_{w}")
            for h in range(WT // XT):
                nc.gpsimd.partition_broadcast(bc[:, h * XT:(h + 1) * XT],
                                              RCS[b][w * (WT // XT) + h][:])
            BC[b].append(bc)

        def emit_epmul(b, w):
            sl = slice(w * WT, (w + 1) * WT)
            for km in range(KM):
                nc.vector.tensor_tensor(EP[b][km][:, sl], E[b][km][:, sl],
                                        BC[b][w][:], op=ALU.mult)

        def emit_consume_co(b, co):
            # all 8 j tiles of one co block: MM2 (j-paired, km outer for
            # stationary reuse) + residual add; stores per co-half
            for jp in range(NJ // 2):
                pos = [ps_o.tile([128, NT], F32, tag="po", name=f"po{b}_{co}_{j}")
                       for j in (2 * jp, 2 * jp + 1)]
                for km in range(KM):
                    for k, j in enumerate((2 * jp, 2 * jp + 1)):
                        nc.tensor.matmul(
                            pos[k][:],
                            WVP[b][km][:, co * 128:(co + 1) * 128],
                            EP[b][km][:, j * NT:(j + 1) * NT],
                            start=(km == 0), stop=(km == KM - 1))
                for k, j in enumerate((2 * jp, 2 * jp + 1)):
                    sl = slice(j * NT, (j + 1) * NT)
                    ys = xs(b, co, sl)
                    if ev_idx[0] % 2 == 0:
                        nc.vector.tensor_tensor(ys, pos[k][:], ys, op=ALU.add)
                    else:
                        t = evpool.tile([128, NT], BF16, tag="ev",
                                        name=f"ev{b}_{co}_{j}")
                        nc.scalar.activation(t[:], pos[k][:], AF.Copy)
                        nc.vector.tensor_tensor(ys, t[:], ys, op=ALU.add)
                    ev_idx[0] += 1
                if jp == 1 or jp == 3:
                    h = jp // 2
                    nc.sync.dma_start(
                        y_d[b, co * 128:(co + 1) * 128, h * (N // 2):(h + 1) * (N // 2)],
                        xs(b, co, slice(h * (N // 2), (h + 1) * (N // 2))))

        # ---- program ----
        init_batch(0)
        init_batch(1)
        for jj in range(NJ2):
            emit_A(0, jj)
        emit_stats(0)
        for j2 in range(NJ2):
            emit_cs(0, j2)
            emit_A(1, j2)
        for w in range(NW):
            emit_bcast(0, w)
            emit_epmul(0, w)
        emit_consume_co(0, 0)
        emit_stats(1)
        emit_cs(1, 0)
        emit_consume_co(0, 1)
        emit_cs(1, 1)
        emit_consume_co(0, 2)
        emit_cs(1, 2)
        emit_consume_co(0, 3)
        emit_cs(1, 3)
        for w in range(NW):
            emit_bcast(1, w)
            emit_epmul(1, w)
        for co in range(KC):
            emit_consume_co(1, co)
    return nc


_CACHE = {}


def _get_program():
    if "nc" not in _CACHE:
        nc = bacc.Bacc("TRN2", target_bir_lowering=False, debug=False,
                       enable_asserts=True)
        _build(nc)
        nc.compile()
        _CACHE["nc"] = nc
    return _CACHE["nc"]


def _in_maps(x, Wk, Wv):
    x = np.asarray(x, dtype=np.float32)
    xb = np.ascontiguousarray(x).astype(BF16NP)
    wkT = np.ascontiguousarray(np.asarray(Wk, np.float32).T).astype(BF16NP)
    wvT = np.ascontiguousarray(np.asarray(Wv, np.float32).T).astype(BF16NP)
    return [{"x": xb[i * BPC:(i + 1) * BPC], "wkT": wkT, "wvT": wvT}
            for i in range(NCORES)]


def kernel(x, Wk, Wv):
    nc = _get_program()
    res = run_bass_kernel_spmd(nc, _in_maps(x, Wk, Wv), list(range(NCORES)))
    y = np.concatenate([res.results[i]["y"].astype(np.float32)
                        for i in range(NCORES)], axis=0)
    return np.ascontiguousarray(y)
